# revision 22
# baseline (speedup 1.0000x reference)
"""ExplainGNN retrieval kernel for 8 Trainium2 NeuronCores.

Strategy (per sharding hint): shard query edges (rows of the big
edge-distance matrix) across the 8 cores, split at node-segment
boundaries so every node segment lives on exactly one core. Each core:

  - gathers the embedding rows of its edges from a host-compacted table
    (device-side indirect DMA), builds edge features ef' = e_u + e_v in
    K-major layout via PE-transpose accumulation (label side processed
    in two halves to fit SBUF),
  - computes -d2' = 2*ef_n'.ef_l' - |ef_n'|^2 - |ef_l'|^2 via f32r
    matmuls (augmented K-chunk carries the norms as f32r value+residual
    pairs so their precision survives),
  - orientation 1 (query rows on partitions): segmented max over label
    segments via chained tensor_tensor_scan + ap_gather of endpoints,
  - orientation 2 (label rows on partitions, second matmul): segmented
    max over local node segments the same way,
  - sqrt + segment means via small fp32 matmuls with host-built weights
    (means accumulate directly into a [NSEG, BL] PSUM score tile),
  - node-level cdist in plain fp32 (exact; it dominates the score),
  - score = -(ALPHA/T)*d_node + mean terms; top-16 via max8/max_index/
    match_replace; host takes top-10.

Host reassembles per-core [NSEG, BL] score rows into [BN, BL] and the
top-k indices, and patches the (measure-zero) empty-segment edge cases
to match jax segment_max/-inf semantics exactly.
"""

import sys

sys.path.insert(0, "/opt/trn_rl_repo")

from contextlib import ExitStack

import numpy as np

import concourse.bass as bass
import concourse.tile as tile
from concourse import bacc, mybir
from concourse.bass_utils import run_bass_kernel_spmd

F32 = mybir.dt.float32
F32R = mybir.dt.float32r
I16 = mybir.dt.int16
U32 = mybir.dt.uint32

ALPHA = 0.5
T = 0.5
TOPK = 10
NCORES = 8
NEG = -3.0e38
PEN = -1.0e30


def _ceil(a, b):
    return -(-a // b) * b


def wrap16(idx, parts=128):
    """16-partition-wrapped index layout for ap_gather / dma_gather,
    replicated across all partition groups."""
    idx = np.asarray(idx, np.int16)
    n = len(idx)
    assert n % 16 == 0, n
    tbl = np.zeros((16, n // 16), np.int16)
    tbl[np.arange(n) % 16, np.arange(n) // 16] = idx
    return np.tile(tbl, (parts // 16, 1))


def _plan(inputs, ncores=NCORES):
    emb = np.asarray(inputs["embedding"], np.float32)
    node_ids = np.asarray(inputs["node_ids"], np.int64)
    label_ids = np.asarray(inputs["label_ids"], np.int64)
    node_edge = np.asarray(inputs["node_edge"], np.int64)
    label_edge = np.asarray(inputs["label_edge"], np.int64)
    node_seg = np.asarray(inputs["node_seg"], np.int64)
    label_seg = np.asarray(inputs["label_seg"], np.int64)

    NN, D = emb.shape
    EN = node_edge.shape[1]
    EL = label_edge.shape[1]
    BN = node_ids.shape[0]
    BL = label_ids.shape[0]
    assert D % 128 == 0 and EL % 256 == 0

    p = dict(D=D, EN=EN, EL=EL, BN=BN, BL=BL, NC=ncores)

    # ---------------- query-edge split at segment boundaries ----------------
    cuts = [0]
    for k in range(1, ncores):
        c = (EN * k) // ncores
        while 0 < c < EN and node_seg[c] == node_seg[c - 1]:
            c += 1
        cuts.append(min(c, EN))
    cuts.append(EN)
    e0s, e1s = cuts[:-1], cuts[1:]
    gs0s = [0] + [int(node_seg[e]) if e < EN else BN for e in e1s[:-1]]
    gs1s = gs0s[1:] + [BN]

    EN_LOC = max(_ceil(max(e1 - e0 for e0, e1 in zip(e0s, e1s)), 256), 256)
    T_Q = EN_LOC // 128
    NSEG16 = _ceil(max(g1 - g0 for g0, g1 in zip(gs0s, gs1s)), 16)
    assert NSEG16 <= 128
    p.update(EN_LOC=EN_LOC, T_Q=T_Q, NSEG16=NSEG16, gs0s=gs0s, gs1s=gs1s)

    # ---------------- label halves + fixed 512 chunks -----------------------
    seg_start = np.r_[0, 1 + np.nonzero(np.diff(label_seg))[0]]
    seg_end = np.r_[seg_start[1:], EL]
    seg_ids = label_seg[seg_start]          # present label segs (sorted)
    nseg_l = len(seg_ids)
    cnt_l = (seg_end - seg_start).astype(np.float32)
    coef_neigh = (1.0 - ALPHA) * 0.5 / T

    ELH = EL // 2
    assert ELH % 128 == 0
    halves = []      # per half: dict(off, width, chunks=[(c0g, w, s0, npad, gidx_cols)], GWH, l0, wseg, wpad, ridx, gidx1)
    for h in range(2):
        off = h * ELH
        chs = []
        slot = 0
        gcols = []
        for c0 in range(off, off + ELH, 512):
            w = min(512, off + ELH - c0)
            # present segments ending inside [c0, c0+w)
            mask = (seg_end - 1 >= c0) & (seg_end - 1 < c0 + w)
            ends = seg_end[mask] - 1 - c0
            ne = len(ends)
            npad = max(_ceil(ne, 16), 16)
            ends_pad = np.concatenate([ends, np.zeros(npad - ne, np.int64)])
            gcols.append(wrap16(ends_pad))
            chs.append(dict(c0=c0, w=w, s0=slot, ne=ne, npad=npad,
                            segs=np.nonzero(mask)[0]))
            slot += npad
        halves.append(dict(off=off, width=ELH, chunks=chs, GWH=slot,
                           gidx1=np.concatenate(gcols, axis=1)))
    # seg -> (half, slot)
    seg_half = np.zeros(nseg_l, np.int64)
    seg_slot = np.zeros(nseg_l, np.int64)
    for h, H in enumerate(halves):
        for ch in H["chunks"]:
            for r, sidx in enumerate(ch["segs"]):
                seg_half[sidx] = h
                seg_slot[sidx] = ch["s0"] + r
    # label-id regions per half (segments ending in half h form a prefix)
    n_end0 = int((seg_half == 0).sum())
    B_id = int(seg_ids[n_end0 - 1]) + 1 if n_end0 else 0
    regions = [(0, B_id), (B_id, BL)]
    for h, H in enumerate(halves):
        l0, l1 = regions[h]
        wseg = l1 - l0
        wpad = max(_ceil(wseg, 16), 16)
        ridx = np.zeros(wpad, np.int64)
        for sidx in range(nseg_l):
            if seg_half[sidx] == h:
                ridx[seg_ids[sidx] - l0] = seg_slot[sidx]
        H.update(l0=l0, wseg=wseg, wpad=wpad, ridx_w=wrap16(ridx.astype(np.int16)))
    GWmax = max(H["GWH"] for H in halves)
    WPmax = max(H["wpad"] for H in halves)
    p.update(halves=halves, GWmax=GWmax, WPmax=WPmax)

    mask1 = np.zeros(EL, np.float32)
    mask1[seg_start] = NEG

    BLV = _ceil(BL, 64)
    Vdiag = np.zeros((BL, BLV), np.float32)
    Vdiag[seg_ids, seg_ids] = -coef_neigh / cnt_l

    absent_l = np.setdiff1d(np.arange(BL), seg_ids)
    pen = np.zeros(BL, np.float32)
    pen[absent_l] = PEN
    p.update(absent_l=absent_l, BLV=BLV)

    # ---------------- compaction + per-core tables --------------------------
    per_core, UMAXES, core_tmp = [], [], []
    for k in range(ncores):
        e0, e1 = e0s[k], e1s[k]
        gs0, gs1 = gs0s[k], gs1s[k]
        enk = e1 - e0
        qe0 = node_edge[0, e0:e1]
        qe1 = node_edge[1, e0:e1]
        if EN_LOC - enk:
            qe0 = np.r_[qe0, np.full(EN_LOC - enk, qe0[-1] if enk else 0)]
            qe1 = np.r_[qe1, np.full(EN_LOC - enk, qe1[-1] if enk else 0)]
        nid = np.r_[node_ids[gs0:gs1],
                    np.zeros(128 - (gs1 - gs0), np.int64)]
        ids = np.concatenate([label_edge[0], label_edge[1], qe0, qe1, nid,
                              label_ids])
        uniq, inv = np.unique(ids, return_inverse=True)
        UMAXES.append(len(uniq))
        core_tmp.append((e0, e1, gs0, gs1, qe0, qe1, nid, uniq, inv))
    UMAX = _ceil(max(UMAXES), 128)
    assert UMAX <= 32000, UMAX
    p.update(UMAX=UMAX)

    lseg_w = wrap16(label_seg.astype(np.int16))

    for k in range(ncores):
        e0, e1, gs0, gs1, qe0, qe1, nid, uniq, inv = core_tmp[k]
        enk = e1 - e0
        nsegk = gs1 - gs0
        emb_loc = np.zeros((UMAX, D), np.float32)
        emb_loc[: len(uniq)] = emb[uniq]
        n = 0
        le0c = inv[n:n + EL]; n += EL
        le1c = inv[n:n + EL]; n += EL
        qe0c = inv[n:n + EN_LOC]; n += EN_LOC
        qe1c = inv[n:n + EN_LOC]; n += EN_LOC
        nidc = inv[n:n + 128]; n += 128
        lidc = inv[n:n + BL]

        ef_q = emb[qe0] + emb[qe1]
        a2 = np.einsum("ij,ij->i", ef_q, ef_q).astype(np.float32)
        ef_l = emb[label_edge[0]] + emb[label_edge[1]]
        b2 = np.einsum("ij,ij->i", ef_l, ef_l).astype(np.float32)
        n2row = emb[nid[:NSEG16]]
        n2 = np.einsum("ij,ij->i", n2row, n2row).astype(np.float32)
        l2row = emb[label_ids]
        l2 = np.einsum("ij,ij->i", l2row, l2row).astype(np.float32)

        lseg_q = node_seg[e0:e1] - gs0
        q_start = np.r_[0, 1 + np.nonzero(np.diff(lseg_q))[0]] if enk else np.array([], np.int64)
        q_end = np.r_[q_start[1:], enk] if enk else np.array([], np.int64)
        q_ids = lseg_q[q_start] if enk else np.array([], np.int64)
        mask2 = np.zeros(EN_LOC, np.float32)
        if enk:
            mask2[q_start] = NEG
        if enk < EN_LOC:
            mask2[enk] = NEG
        gidx2_flat = np.zeros(NSEG16, np.int64)
        cnt_q = np.zeros(NSEG16, np.float32)
        present_q = np.zeros(NSEG16, bool)
        for s, e, sid in zip(q_start, q_end, q_ids):
            gidx2_flat[sid] = e - 1
            cnt_q[sid] = e - s
            present_q[sid] = True
        W_n = np.zeros((EN_LOC, NSEG16), np.float32)
        if enk:
            W_n[np.arange(enk), lseg_q] = -coef_neigh / cnt_q[lseg_q]

        im = {
            "emb_loc": emb_loc,
            "vdiag": Vdiag,
            "le0": wrap16(le0c.astype(np.int16)),
            "le1": wrap16(le1c.astype(np.int16)),
            "qe0": wrap16(qe0c.astype(np.int16)),
            "qe1": wrap16(qe1c.astype(np.int16)),
            "nid": wrap16(nidc.astype(np.int16)),
            "lid": wrap16(lidc.astype(np.int16)),
            "lsegw": lseg_w,
            "gidx1h0": halves[0]["gidx1"],
            "gidx1h1": halves[1]["gidx1"],
            "ridxh0": halves[0]["ridx_w"],
            "ridxh1": halves[1]["ridx_w"],
            "gidx2": wrap16(gidx2_flat.astype(np.int16)),
            "mask1": np.tile(mask1[None, :], (128, 1)),
            "mask2": np.tile(mask2[None, :], (128, 1)),
            "a3": np.stack([a2, np.ones(EN_LOC, np.float32)]),
            "b3": np.stack([-b2, -np.ones(EL, np.float32)]),
            "an3": np.stack([n2, np.ones(NSEG16, np.float32)]),
            "bl3": np.stack([-np.ones(BL, np.float32), -l2]),
            "pen": pen[None, :],
            "ones1": np.ones((1, NSEG16), np.float32),
            "wn": W_n,
            "ident": np.eye(128, dtype=np.float32),
        }
        per_core.append(dict(inputs=im, gs0=gs0, gs1=gs1, nsegk=nsegk,
                             present_q=present_q))
    p["per_core"] = per_core
    return p


def _build(p):
    D, EL, BL, BLV = p["D"], p["EL"], p["BL"], p["BLV"]
    EN_LOC, T_Q, NSEG16, UMAX = p["EN_LOC"], p["T_Q"], p["NSEG16"], p["UMAX"]
    D2 = D // 128
    ELH = EL // 2
    ELT_H = ELH // 128
    halves = p["halves"]
    GWmax, WPmax = p["GWmax"], p["WPmax"]

    nc = bacc.Bacc("TRN2", target_bir_lowering=False, debug=False,
                   num_devices=p["NC"])

    def din(name, shape, dt=F32):
        return nc.dram_tensor(name, shape, dt, kind="ExternalInput")

    emb_d = din("emb_loc", [UMAX, D])
    vdiag_d = din("vdiag", [BL, BLV])
    idx_d = {n: din(n, [128, s // 16], I16) for n, s in [
        ("le0", EL), ("le1", EL), ("qe0", EN_LOC), ("qe1", EN_LOC),
        ("nid", 128), ("lid", BL), ("lsegw", EL), ("gidx2", NSEG16)]}
    gidx1_d = [din(f"gidx1h{h}", [128, halves[h]["GWH"] // 16], I16) for h in range(2)]
    ridx_d = [din(f"ridxh{h}", [128, halves[h]["wpad"] // 16], I16) for h in range(2)]
    mask1_d = din("mask1", [128, EL])
    mask2_d = din("mask2", [128, EN_LOC])
    a3_d = din("a3", [2, EN_LOC])
    b3_d = din("b3", [2, EL])
    an3_d = din("an3", [2, NSEG16])
    bl3_d = din("bl3", [2, BL])
    pen_d = din("pen", [1, BL])
    ones1_d = din("ones1", [1, NSEG16])
    wn_d = din("wn", [EN_LOC, NSEG16])
    ident_d = din("ident", [128, 128])

    score_d = nc.dram_tensor("score", [NSEG16, BL], F32, kind="ExternalOutput")
    tidx_d = nc.dram_tensor("tidx", [NSEG16, 16], U32, kind="ExternalOutput")

    with tile.TileContext(nc) as tc, ExitStack() as ctx:
        const = ctx.enter_context(tc.tile_pool(name="const", bufs=1))
        hpool = ctx.enter_context(tc.tile_pool(name="hpool", bufs=1))
        work = ctx.enter_context(tc.tile_pool(name="work", bufs=2))
        ps1 = ctx.enter_context(tc.tile_pool(name="ps1", bufs=2, space="PSUM"))
        ps2 = ctx.enter_context(tc.tile_pool(name="ps2", bufs=2, space="PSUM"))
        psacc = ctx.enter_context(tc.tile_pool(name="psacc", bufs=1, space="PSUM"))

        # ---------------- constants / tables ----------------
        ident_t = const.tile([128, 128], F32, name="ident_t")
        nc.sync.dma_start(ident_t[:], ident_d.ap())
        mask2_t = const.tile([128, EN_LOC], F32, name="mask2_t")
        nc.sync.dma_start(mask2_t[:], mask2_d.ap())
        idx_t = {}
        for n, d_ in idx_d.items():
            idx_t[n] = const.tile(list(d_.shape), I16, name=f"{n}_t")
            nc.sync.dma_start(idx_t[n][:], d_.ap())
        gidx1_t = []
        ridx_t = []
        for h in range(2):
            g = const.tile(list(gidx1_d[h].shape), I16, name=f"gidx1h{h}_t")
            nc.sync.dma_start(g[:], gidx1_d[h].ap())
            gidx1_t.append(g)
            r = const.tile(list(ridx_d[h].shape), I16, name=f"ridxh{h}_t")
            nc.sync.dma_start(r[:], ridx_d[h].ap())
            ridx_t.append(r)
        pen_t = const.tile([1, BL], F32, name="pen_t")
        nc.sync.dma_start(pen_t[:], pen_d.ap())
        ones1_t = const.tile([1, NSEG16], F32, name="ones1_t")
        nc.sync.dma_start(ones1_t[:], ones1_d.ap())
        an3_t = const.tile([2, NSEG16], F32, name="an3_t")
        nc.sync.dma_start(an3_t[:], an3_d.ap())
        bl3_t = const.tile([2, BL], F32, name="bl3_t")
        nc.sync.dma_start(bl3_t[:], bl3_d.ap())

        # Augmented-row builder: dst rows get [value_r; value_res] at vrows
        # and +-ones at orows (value = src row0, ones = src row1).  The f32r
        # rounding happens in a [128, X] wide layout (fast), the residual is
        # staged back row-major, and one whole-tile f32r copy per chunk is
        # the rounding producer walrus wants.
        def build_aug(dst, src_d, width, vrows, orows, soff=0):
            """dst[4, width] f32r gets [value_r; value_res] at vrows and
            +-ones at orows, from src_d rows [value; ones] cols
            [soff, soff+width).  Rounding runs in a [128, X] wide layout;
            the whole-tile f32r copy per chunk is the rounding producer."""
            for c0i in range(0, width, 2048):
                cw = min(2048, width - c0i)
                assert cw % 128 == 0
                X = cw // 128
                c0 = c0i + soff
                wide_ap = src_d.ap()[0:1, c0:c0 + cw].rearrange(
                    "o (p x) -> (o p) x", p=128)
                vr = work.tile([128, 16], F32, name="vr", tag="augv", bufs=2)
                nc.sync.dma_start(vr[:, 0:X], wide_ap)
                vt1 = work.tile([128, 16], F32R, name="vt1", tag="augt", bufs=2)
                nc.vector.tensor_copy(vt1[:, 0:X], vr[:, 0:X])
                vres = work.tile([128, 16], F32, name="vres", tag="augr", bufs=2)
                nc.vector.tensor_sub(vres[:, 0:X], vr[:, 0:X],
                                     vt1[:, 0:X].bitcast(F32))
                stg = work.tile([4, 2048], F32, name="stg", tag="augstg", bufs=1)
                nc.sync.dma_start(stg[vrows[0]:vrows[0] + 1, 0:cw],
                                  src_d.ap()[0:1, c0:c0 + cw])
                nc.sync.dma_start(stg[vrows[1]:vrows[1] + 1, 0:cw],
                                  vres[:, 0:X])
                for r_ in orows:
                    nc.sync.dma_start(stg[r_:r_ + 1, 0:cw],
                                      src_d.ap()[1:2, c0:c0 + cw])
                nc.vector.tensor_copy(dst[:, c0i:c0i + cw], stg[:, 0:cw])

        # A side: [a2_r; a2_res; 1; 1]
        a3r_t = const.tile([4, EN_LOC], F32R, name="a3_r")
        build_aug(a3r_t, a3_d, EN_LOC, vrows=(0, 1), orows=(2, 3))

        # ---------------- K-major features ----------------
        ak_ts = [const.tile([128, EN_LOC], F32R, name=f"ak{d}") for d in range(D2)]
        ank_ts = [const.tile([128, 128], F32, name=f"ank{d}") for d in range(D2)]
        blk_ts = [const.tile([128, BL], F32, name=f"blk{d}") for d in range(D2)]

        def build_kmajor(idx0, idx1, n_idx, dsts, scale, single=False,
                         idx_off=0, col_off=0, nb=256):
            for ch0 in range(0, n_idx, nb):
                w = min(nb, n_idx - ch0)
                nsub = max(w // 128, 1)
                g0 = work.tile([128, nsub, D], F32, name="g0", tag="gbuf0")
                nc.gpsimd.dma_gather(
                    g0[:], emb_d.ap(),
                    idx0[:, (idx_off + ch0) // 16:(idx_off + ch0 + w) // 16],
                    w, w, D)
                if not single:
                    g1 = work.tile([128, nsub, D], F32, name="g1", tag="gbuf1")
                    nc.gpsimd.dma_gather(
                        g1[:], emb_d.ap(),
                        idx1[:, (idx_off + ch0) // 16:(idx_off + ch0 + w) // 16],
                        w, w, D)
                for s in range(nsub):
                    for d_ in range(D2):
                        pst = ps1.tile([128, 128], F32, name="pst", tag="ps1")
                        nc.tensor.matmul(pst[:], g0[:, s, d_ * 128:(d_ + 1) * 128],
                                         ident_t[:], is_transpose=True,
                                         start=True, stop=single)
                        if not single:
                            nc.tensor.matmul(pst[:], g1[:, s, d_ * 128:(d_ + 1) * 128],
                                             ident_t[:], is_transpose=True,
                                             start=False, stop=True)
                        col0 = col_off + ch0 + s * 128
                        cw = min(128, col_off + n_idx - col0)
                        nc.scalar.mul(dsts[d_][:, col0:col0 + cw], pst[:, 0:cw], scale)

        assert NSEG16 <= 128
        build_kmajor(idx_t["qe0"], idx_t["qe1"], EN_LOC, ak_ts, 1.0)
        build_kmajor(idx_t["nid"], None, 128, ank_ts, 1.0, single=True)
        build_kmajor(idx_t["lid"], None, BL, blk_ts, 2.0, single=True)

        # score accumulation PSUM [NSEG16, BL]
        ps_mean = psacc.tile([NSEG16, BL], F32, name="ps_mean")
        ps_node = psacc.tile([NSEG16, BL], F32, name="ps_node")

        keep_t = const.tile([128, T_Q], F32, name="keep_t")

        # open the ps_mean accumulation with the absent-label penalty row
        # (start=True zeroes the whole [NSEG16, BL] region; everything else
        # accumulates onto it in Tile-tracked emission order)
        nc.tensor.matmul(ps_mean[:], ones1_t[:], pen_t[:], start=True,
                         stop=False, skip_group_check=True)

        # ---------------- main: two label halves ----------------
        for h, H in enumerate(halves):
            off, chs = H["off"], H["chunks"]
            # B-side K-major for this half
            bk_ts = [hpool.tile([128, ELH], F32R, name=f"bk{d}", tag=f"bk{d}")
                     for d in range(D2)]
            build_kmajor(idx_t["le0"], idx_t["le1"], ELH, bk_ts, 2.0,
                         idx_off=off)
            # mask half
            m1 = hpool.tile([128, ELH], F32, name="m1", tag="m1")
            nc.sync.dma_start(m1[:], mask1_d.ap()[:, off:off + ELH])
            # B-side augmented rows for this half: [-1; -1; -b2_r; -b2_res]
            b3r_t = hpool.tile([4, ELH], F32R, name="b3_r", tag="b3r")
            build_aug(b3r_t, b3_d, ELH, vrows=(2, 3), orows=(0, 1), soff=off)

            # ---- orientation 1 ----
            l0, wseg, wpad = H["l0"], H["wseg"], H["wpad"]
            for t in range(T_Q):
                strip = work.tile([128, GWmax], F32, name="strip", tag="strip")
                prev = None
                for ch in chs:
                    c0l = ch["c0"] - off
                    w, s0, npad = ch["w"], ch["s0"], ch["npad"]
                    ps = ps1.tile([128, 512], F32, name="pso1", tag="ps1")
                    for d_ in range(D2):
                        nc.tensor.matmul(ps[:, 0:w],
                                         ak_ts[d_][:, t * 128:(t + 1) * 128],
                                         bk_ts[d_][:, c0l:c0l + w],
                                         start=(d_ == 0), stop=False)
                    nc.tensor.matmul(ps[:, 0:w], a3r_t[:, t * 128:(t + 1) * 128],
                                     b3r_t[:, c0l:c0l + w], start=False, stop=True)
                    sc = work.tile([128, 512], F32, name="sc1", tag="sc1")
                    if prev is None:
                        init = keep_t[:, t:t + 1] if h == 1 else NEG
                    else:
                        init = prev[:, prev_w - 1:prev_w]
                    nc.vector.tensor_tensor_scan(
                        sc[:, 0:w], m1[:, c0l:c0l + w], ps[:, 0:w], initial=init,
                        op0=mybir.AluOpType.add, op1=mybir.AluOpType.max)
                    nc.gpsimd.ap_gather(
                        strip[:, s0:s0 + npad], sc[:, 0:w],
                        gidx1_t[h][:, s0 // 16:(s0 + npad) // 16],
                        channels=128, num_elems=w, d=1, num_idxs=npad)
                    prev, prev_w = sc, w
                if h == 0:
                    nc.vector.tensor_copy(keep_t[:, t:t + 1],
                                          prev[:, prev_w - 1:prev_w])
                # slots -> label-id columns, clamp+sqrt, mean matmul
                go = work.tile([128, WPmax], F32, name="go1", tag="go1")
                nc.gpsimd.ap_gather(go[:, 0:wpad], strip[:, 0:H["GWH"]],
                                    ridx_t[h][:], channels=128,
                                    num_elems=H["GWH"], d=1, num_idxs=wpad)
                nc.vector.tensor_scalar(go[:, 0:wseg], go[:, 0:wseg], -0.25,
                                        1e-12, op0=mybir.AluOpType.mult,
                                        op1=mybir.AluOpType.max)
                nc.scalar.sqrt(go[:, 0:wseg], go[:, 0:wseg])
                wn = work.tile([128, NSEG16], F32, name="wn", tag="wn")
                nc.sync.dma_start(wn[:], wn_d.ap()[t * 128:(t + 1) * 128, :])
                for n0 in range(0, wseg, 512):
                    nw = min(512, wseg - n0)
                    nc.tensor.matmul(ps_mean[:, l0 + n0:l0 + n0 + nw], wn[:],
                                     go[:, n0:n0 + nw], start=False,
                                     stop=False, skip_group_check=True)

            # ---- orientation 2 ----
            HQ = EN_LOC // 2
            for m in range(ELT_H):
                sc2 = work.tile([128, EN_LOC], F32, name="sc2", tag="sc2")
                for hq in range(2):
                    psb = ps2.tile([128, HQ], F32, name="pso2", tag="ps2")
                    h0 = hq * HQ
                    for n0 in range(0, HQ, 512):
                        nw = min(512, HQ - n0)
                        for d_ in range(D2):
                            nc.tensor.matmul(psb[:, n0:n0 + nw],
                                             bk_ts[d_][:, m * 128:(m + 1) * 128],
                                             ak_ts[d_][:, h0 + n0:h0 + n0 + nw],
                                             start=(d_ == 0), stop=False)
                        nc.tensor.matmul(psb[:, n0:n0 + nw],
                                         b3r_t[:, m * 128:(m + 1) * 128],
                                         a3r_t[:, h0 + n0:h0 + n0 + nw],
                                         start=False, stop=True)
                    nc.vector.tensor_tensor_scan(
                        sc2[:, h0:h0 + HQ], mask2_t[:, h0:h0 + HQ], psb[:],
                        initial=(NEG if hq == 0 else sc2[:, h0 - 1:h0]),
                        op0=mybir.AluOpType.add, op1=mybir.AluOpType.max)
                g2 = work.tile([128, NSEG16], F32, name="g2", tag="g2")
                nc.gpsimd.ap_gather(g2[:], sc2[:, 0:EN_LOC], idx_t["gidx2"][:],
                                    channels=128, num_elems=EN_LOC, d=1,
                                    num_idxs=NSEG16)
                nc.vector.tensor_scalar(g2[:], g2[:], -0.25, 1e-12,
                                        op0=mybir.AluOpType.mult,
                                        op1=mybir.AluOpType.max)
                nc.scalar.sqrt(g2[:], g2[:])
                wl = work.tile([128, 1, BLV], F32, name="wl", tag="wl")
                mg = off // 128 + m
                nc.gpsimd.dma_gather(wl[:], vdiag_d.ap(),
                                     idx_t["lsegw"][:, mg * 8:(mg + 1) * 8],
                                     128, 128, BLV)
                nc.tensor.matmul(ps_mean[:], g2[:], wl[:, 0, 0:BL],
                                 start=False,
                                 stop=(h == 1 and m == ELT_H - 1),
                                 skip_group_check=True)

        # ---------------- node-level score + combine + topk ----------------
        for d_ in range(D2):
            nc.tensor.matmul(ps_node[:], ank_ts[d_][:, 0:NSEG16], blk_ts[d_][:],
                             start=(d_ == 0), stop=False)
        nc.tensor.matmul(ps_node[:], an3_t[:], bl3_t[:], start=False, stop=True)
        nd = work.tile([NSEG16, BL], F32, name="nd", bufs=1)
        nc.vector.tensor_scalar(nd[:], ps_node[:], -1.0, 1e-12,
                                op0=mybir.AluOpType.mult, op1=mybir.AluOpType.max)
        nc.scalar.sqrt(nd[:], nd[:])
        score_t = work.tile([NSEG16, BL], F32, name="score_t", bufs=1)
        nc.vector.scalar_tensor_tensor(
            score_t[:], nd[:], -(ALPHA / T), ps_mean[:],
            op0=mybir.AluOpType.mult, op1=mybir.AluOpType.add)
        nc.sync.dma_start(score_d.ap(), score_t[:])

        top_t = work.tile([NSEG16, 16], F32, name="top_t", bufs=1)
        tix_t = work.tile([NSEG16, 16], U32, name="tix_t", bufs=1)
        sc_copy = work.tile([NSEG16, BL], F32, name="sc_copy", bufs=1)
        nc.vector.tensor_copy(sc_copy[:], score_t[:])
        nc.vector.max(top_t[:, 0:8], sc_copy[:])
        nc.vector.max_index(tix_t[:, 0:8], top_t[:, 0:8], sc_copy[:])
        nc.vector.match_replace(sc_copy[:], top_t[:, 0:8], sc_copy[:], NEG)
        nc.vector.max(top_t[:, 8:16], sc_copy[:])
        nc.vector.max_index(tix_t[:, 8:16], top_t[:, 8:16], sc_copy[:])
        nc.sync.dma_start(tidx_d.ap(), tix_t[:])

    nc.compile()
    return nc


def kernel(**inputs):
    p = _plan(inputs)
    nc = _build(p)
    in_maps = [c["inputs"] for c in p["per_core"]]
    res = run_bass_kernel_spmd(nc, in_maps, list(range(p["NC"])))
    return _assemble(p, [res.results[k] for k in range(p["NC"])])


def _assemble(p, results):
    BN, BL = p["BN"], p["BL"]
    score = np.zeros((BN, BL), np.float32)
    tidx = np.zeros((BN, TOPK), np.int32)
    for k, c in enumerate(p["per_core"]):
        r = results[k]
        n = c["nsegk"]
        score[c["gs0"]:c["gs1"]] = r["score"][:n]
        tidx[c["gs0"]:c["gs1"]] = r["tidx"][:n, :TOPK].astype(np.int32)
        pq = c["present_q"][:n]
        if not pq.all():
            rows = np.nonzero(~pq)[0] + c["gs0"]
            score[rows] = -np.inf
            tidx[rows] = np.arange(TOPK)[None, :]
    if len(p["absent_l"]):
        score[:, p["absent_l"]] = -np.inf
    return score, tidx


if __name__ == "__main__":
    sys.path.insert(0, "/root/problem")
    import reference

    inputs = {k: np.asarray(v) for k, v in reference.setup_inputs().items()}
    out = kernel(**inputs)
    print("score", out[0].shape, "tidx", out[1].shape)


# revision 23
# speedup vs baseline: 1.3836x; 1.3836x over previous
"""ExplainGNN retrieval kernel for 8 Trainium2 NeuronCores.

Strategy (per sharding hint): shard query edges (rows of the big
edge-distance matrix) across the 8 cores, split at node-segment
boundaries so every node segment lives on exactly one core. Each core:

  - gathers the embedding rows of its edges from a host-compacted table
    (device-side indirect DMA), builds edge features ef' = e_u + e_v in
    K-major layout via PE-transpose accumulation (label side processed
    in two halves to fit SBUF),
  - computes -d2' = 2*ef_n'.ef_l' - |ef_n'|^2 - |ef_l'|^2 via f32r
    matmuls (augmented K-chunk carries the norms as f32r value+residual
    pairs so their precision survives),
  - orientation 1 (query rows on partitions): segmented max over label
    segments via chained tensor_tensor_scan + ap_gather of endpoints,
  - orientation 2 (label rows on partitions, second matmul): segmented
    max over local node segments the same way,
  - sqrt + segment means via small fp32 matmuls with host-built weights
    (means accumulate directly into a [NSEG, BL] PSUM score tile),
  - node-level cdist in plain fp32 (exact; it dominates the score),
  - score = -(ALPHA/T)*d_node + mean terms; top-16 via max8/max_index/
    match_replace; host takes top-10.

Host reassembles per-core [NSEG, BL] score rows into [BN, BL] and the
top-k indices, and patches the (measure-zero) empty-segment edge cases
to match jax segment_max/-inf semantics exactly.
"""

import sys

sys.path.insert(0, "/opt/trn_rl_repo")

from contextlib import ExitStack

import numpy as np

import concourse.bass as bass
import concourse.tile as tile
from concourse import bacc, mybir
from concourse.bass_utils import run_bass_kernel_spmd

F32 = mybir.dt.float32
F32R = mybir.dt.float32r
I16 = mybir.dt.int16
U32 = mybir.dt.uint32

ALPHA = 0.5
T = 0.5
TOPK = 10
NCORES = 8
NEG = -3.0e38
PEN = -1.0e30


def _ceil(a, b):
    return -(-a // b) * b


def wrap16(idx, parts=128):
    """16-partition-wrapped index layout for ap_gather / dma_gather,
    replicated across all partition groups."""
    idx = np.asarray(idx, np.int16)
    n = len(idx)
    assert n % 16 == 0, n
    tbl = np.zeros((16, n // 16), np.int16)
    tbl[np.arange(n) % 16, np.arange(n) // 16] = idx
    return np.tile(tbl, (parts // 16, 1))


def _plan(inputs, ncores=NCORES):
    emb = np.asarray(inputs["embedding"], np.float32)
    node_ids = np.asarray(inputs["node_ids"], np.int64)
    label_ids = np.asarray(inputs["label_ids"], np.int64)
    node_edge = np.asarray(inputs["node_edge"], np.int64)
    label_edge = np.asarray(inputs["label_edge"], np.int64)
    node_seg = np.asarray(inputs["node_seg"], np.int64)
    label_seg = np.asarray(inputs["label_seg"], np.int64)

    NN, D = emb.shape
    EN = node_edge.shape[1]
    EL = label_edge.shape[1]
    BN = node_ids.shape[0]
    BL = label_ids.shape[0]
    assert D % 128 == 0 and EL % 256 == 0

    p = dict(D=D, EN=EN, EL=EL, BN=BN, BL=BL, NC=ncores)

    # ---------------- query-edge split at segment boundaries ----------------
    cuts = [0]
    for k in range(1, ncores):
        c = (EN * k) // ncores
        while 0 < c < EN and node_seg[c] == node_seg[c - 1]:
            c += 1
        cuts.append(min(c, EN))
    cuts.append(EN)
    e0s, e1s = cuts[:-1], cuts[1:]
    gs0s = [0] + [int(node_seg[e]) if e < EN else BN for e in e1s[:-1]]
    gs1s = gs0s[1:] + [BN]

    EN_LOC = max(_ceil(max(e1 - e0 for e0, e1 in zip(e0s, e1s)), 256), 256)
    T_Q = EN_LOC // 128
    NSEG16 = _ceil(max(g1 - g0 for g0, g1 in zip(gs0s, gs1s)), 16)
    assert NSEG16 <= 128
    p.update(EN_LOC=EN_LOC, T_Q=T_Q, NSEG16=NSEG16, gs0s=gs0s, gs1s=gs1s)

    # ---------------- label halves + fixed 512 chunks -----------------------
    seg_start = np.r_[0, 1 + np.nonzero(np.diff(label_seg))[0]]
    seg_end = np.r_[seg_start[1:], EL]
    seg_ids = label_seg[seg_start]          # present label segs (sorted)
    nseg_l = len(seg_ids)
    cnt_l = (seg_end - seg_start).astype(np.float32)
    coef_neigh = (1.0 - ALPHA) * 0.5 / T

    ELH = EL // 2
    assert ELH % 128 == 0
    halves = []      # per half: dict(off, width, chunks=[(c0g, w, s0, npad, gidx_cols)], GWH, l0, wseg, wpad, ridx, gidx1)
    for h in range(2):
        off = h * ELH
        chs = []
        slot = 0
        gcols = []
        for c0 in range(off, off + ELH, 512):
            w = min(512, off + ELH - c0)
            # present segments ending inside [c0, c0+w)
            mask = (seg_end - 1 >= c0) & (seg_end - 1 < c0 + w)
            ends = seg_end[mask] - 1 - c0
            ne = len(ends)
            npad = max(_ceil(ne, 16), 16)
            ends_pad = np.concatenate([ends, np.zeros(npad - ne, np.int64)])
            gcols.append(wrap16(ends_pad))
            chs.append(dict(c0=c0, w=w, s0=slot, ne=ne, npad=npad,
                            segs=np.nonzero(mask)[0]))
            slot += npad
        halves.append(dict(off=off, width=ELH, chunks=chs, GWH=slot,
                           gidx1=np.concatenate(gcols, axis=1)))
    # seg -> (half, slot)
    seg_half = np.zeros(nseg_l, np.int64)
    seg_slot = np.zeros(nseg_l, np.int64)
    for h, H in enumerate(halves):
        for ch in H["chunks"]:
            for r, sidx in enumerate(ch["segs"]):
                seg_half[sidx] = h
                seg_slot[sidx] = ch["s0"] + r
    # label-id regions per half (segments ending in half h form a prefix)
    n_end0 = int((seg_half == 0).sum())
    B_id = int(seg_ids[n_end0 - 1]) + 1 if n_end0 else 0
    regions = [(0, B_id), (B_id, BL)]
    for h, H in enumerate(halves):
        l0, l1 = regions[h]
        wseg = l1 - l0
        wpad = max(_ceil(wseg, 16), 16)
        ridx = np.zeros(wpad, np.int64)
        for sidx in range(nseg_l):
            if seg_half[sidx] == h:
                ridx[seg_ids[sidx] - l0] = seg_slot[sidx]
        H.update(l0=l0, wseg=wseg, wpad=wpad, ridx_w=wrap16(ridx.astype(np.int16)))
    GWmax = max(H["GWH"] for H in halves)
    WPmax = max(H["wpad"] for H in halves)
    p.update(halves=halves, GWmax=GWmax, WPmax=WPmax)

    mask1 = np.zeros(EL, np.float32)
    mask1[seg_start] = NEG

    BLV = _ceil(BL, 64)
    Vdiag = np.zeros((BL, BLV), np.float32)
    Vdiag[seg_ids, seg_ids] = -coef_neigh / cnt_l

    absent_l = np.setdiff1d(np.arange(BL), seg_ids)
    pen = np.zeros(BL, np.float32)
    pen[absent_l] = PEN
    p.update(absent_l=absent_l, BLV=BLV)

    # ---------------- compaction + per-core tables --------------------------
    per_core, UMAXES, core_tmp = [], [], []
    for k in range(ncores):
        e0, e1 = e0s[k], e1s[k]
        gs0, gs1 = gs0s[k], gs1s[k]
        enk = e1 - e0
        qe0 = node_edge[0, e0:e1]
        qe1 = node_edge[1, e0:e1]
        if EN_LOC - enk:
            qe0 = np.r_[qe0, np.full(EN_LOC - enk, qe0[-1] if enk else 0)]
            qe1 = np.r_[qe1, np.full(EN_LOC - enk, qe1[-1] if enk else 0)]
        nid = np.r_[node_ids[gs0:gs1],
                    np.zeros(128 - (gs1 - gs0), np.int64)]
        ids = np.concatenate([label_edge[0], label_edge[1], qe0, qe1, nid,
                              label_ids])
        uniq, inv = np.unique(ids, return_inverse=True)
        UMAXES.append(len(uniq))
        core_tmp.append((e0, e1, gs0, gs1, qe0, qe1, nid, uniq, inv))
    UMAX = _ceil(max(UMAXES), 128)
    assert UMAX <= 32000, UMAX
    p.update(UMAX=UMAX)

    lseg_w = wrap16(label_seg.astype(np.int16))

    for k in range(ncores):
        e0, e1, gs0, gs1, qe0, qe1, nid, uniq, inv = core_tmp[k]
        enk = e1 - e0
        nsegk = gs1 - gs0
        emb_loc = np.zeros((UMAX, D), np.float32)
        emb_loc[: len(uniq)] = emb[uniq]
        n = 0
        le0c = inv[n:n + EL]; n += EL
        le1c = inv[n:n + EL]; n += EL
        qe0c = inv[n:n + EN_LOC]; n += EN_LOC
        qe1c = inv[n:n + EN_LOC]; n += EN_LOC
        nidc = inv[n:n + 128]; n += 128
        lidc = inv[n:n + BL]

        ef_q = emb[qe0] + emb[qe1]
        a2 = np.einsum("ij,ij->i", ef_q, ef_q).astype(np.float32)
        ef_l = emb[label_edge[0]] + emb[label_edge[1]]
        b2 = np.einsum("ij,ij->i", ef_l, ef_l).astype(np.float32)
        n2row = emb[nid[:NSEG16]]
        n2 = np.einsum("ij,ij->i", n2row, n2row).astype(np.float32)
        l2row = emb[label_ids]
        l2 = np.einsum("ij,ij->i", l2row, l2row).astype(np.float32)

        lseg_q = node_seg[e0:e1] - gs0
        q_start = np.r_[0, 1 + np.nonzero(np.diff(lseg_q))[0]] if enk else np.array([], np.int64)
        q_end = np.r_[q_start[1:], enk] if enk else np.array([], np.int64)
        q_ids = lseg_q[q_start] if enk else np.array([], np.int64)
        mask2 = np.zeros(EN_LOC, np.float32)
        if enk:
            mask2[q_start] = NEG
        if enk < EN_LOC:
            mask2[enk] = NEG
        gidx2_flat = np.zeros(NSEG16, np.int64)
        cnt_q = np.zeros(NSEG16, np.float32)
        present_q = np.zeros(NSEG16, bool)
        for s, e, sid in zip(q_start, q_end, q_ids):
            gidx2_flat[sid] = e - 1
            cnt_q[sid] = e - s
            present_q[sid] = True
        W_n = np.zeros((EN_LOC, NSEG16), np.float32)
        if enk:
            W_n[np.arange(enk), lseg_q] = -coef_neigh / cnt_q[lseg_q]

        im = {
            "emb_loc": emb_loc,
            "vdiag": Vdiag,
            "le0": wrap16(le0c.astype(np.int16)),
            "le1": wrap16(le1c.astype(np.int16)),
            "qe0": wrap16(qe0c.astype(np.int16)),
            "qe1": wrap16(qe1c.astype(np.int16)),
            "nid": wrap16(nidc.astype(np.int16)),
            "lid": wrap16(lidc.astype(np.int16)),
            "lsegw": lseg_w,
            "gidx1h0": halves[0]["gidx1"],
            "gidx1h1": halves[1]["gidx1"],
            "ridxh0": halves[0]["ridx_w"],
            "ridxh1": halves[1]["ridx_w"],
            "gidx2": wrap16(gidx2_flat.astype(np.int16)),
            "mask1": np.tile(mask1[None, :], (128, 1)),
            "mask2": np.tile(mask2[None, :], (128, 1)),
            "a3": np.stack([a2, np.ones(EN_LOC, np.float32)]),
            "b3": np.stack([-b2, -np.ones(EL, np.float32)]),
            "an3": np.stack([n2, np.ones(NSEG16, np.float32)]),
            "bl3": np.stack([-np.ones(BL, np.float32), -l2]),
            "pen": pen[None, :],
            "ones1": np.ones((1, NSEG16), np.float32),
            "wn": W_n,
            "ident": np.eye(128, dtype=np.float32),
        }
        per_core.append(dict(inputs=im, gs0=gs0, gs1=gs1, nsegk=nsegk,
                             present_q=present_q))
    p["per_core"] = per_core
    return p


def _build(p, reps=1):
    D, EL, BL, BLV = p["D"], p["EL"], p["BL"], p["BLV"]
    EN_LOC, T_Q, NSEG16, UMAX = p["EN_LOC"], p["T_Q"], p["NSEG16"], p["UMAX"]
    D2 = D // 128
    ELH = EL // 2
    ELT_H = ELH // 128
    halves = p["halves"]
    GWmax, WPmax = p["GWmax"], p["WPmax"]

    nc = bacc.Bacc("TRN2", target_bir_lowering=False, debug=False,
                   num_devices=p["NC"])

    def din(name, shape, dt=F32):
        return nc.dram_tensor(name, shape, dt, kind="ExternalInput")

    emb_d = din("emb_loc", [UMAX, D])
    vdiag_d = din("vdiag", [BL, BLV])
    idx_d = {n: din(n, [128, s // 16], I16) for n, s in [
        ("le0", EL), ("le1", EL), ("qe0", EN_LOC), ("qe1", EN_LOC),
        ("nid", 128), ("lid", BL), ("lsegw", EL), ("gidx2", NSEG16)]}
    gidx1_d = [din(f"gidx1h{h}", [128, halves[h]["GWH"] // 16], I16) for h in range(2)]
    ridx_d = [din(f"ridxh{h}", [128, halves[h]["wpad"] // 16], I16) for h in range(2)]
    mask1_d = din("mask1", [128, EL])
    mask2_d = din("mask2", [128, EN_LOC])
    a3_d = din("a3", [2, EN_LOC])
    b3_d = din("b3", [2, EL])
    an3_d = din("an3", [2, NSEG16])
    bl3_d = din("bl3", [2, BL])
    pen_d = din("pen", [1, BL])
    ones1_d = din("ones1", [1, NSEG16])
    wn_d = din("wn", [EN_LOC, NSEG16])
    ident_d = din("ident", [128, 128])

    score_d = nc.dram_tensor("score", [NSEG16, BL], F32, kind="ExternalOutput")
    tidx_d = nc.dram_tensor("tidx", [NSEG16, 16], U32, kind="ExternalOutput")

    with tile.TileContext(nc) as tc, ExitStack() as ctx:
        const = ctx.enter_context(tc.tile_pool(name="const", bufs=1))
        hpool = ctx.enter_context(tc.tile_pool(name="hpool", bufs=1))
        work = ctx.enter_context(tc.tile_pool(name="work", bufs=2))
        ps1 = ctx.enter_context(tc.tile_pool(name="ps1", bufs=2, space="PSUM"))
        ps2 = ctx.enter_context(tc.tile_pool(name="ps2", bufs=2, space="PSUM"))
        psacc = ctx.enter_context(tc.tile_pool(name="psacc", bufs=1, space="PSUM"))

        # ---------------- constants / tables ----------------
        ident_t = const.tile([128, 128], F32, name="ident_t")
        nc.sync.dma_start(ident_t[:], ident_d.ap())
        mask2_t = const.tile([128, EN_LOC], F32, name="mask2_t")
        nc.sync.dma_start(mask2_t[:], mask2_d.ap())
        idx_t = {}
        for n, d_ in idx_d.items():
            idx_t[n] = const.tile(list(d_.shape), I16, name=f"{n}_t")
            nc.sync.dma_start(idx_t[n][:], d_.ap())
        gidx1_t = []
        ridx_t = []
        for h in range(2):
            g = const.tile(list(gidx1_d[h].shape), I16, name=f"gidx1h{h}_t")
            nc.sync.dma_start(g[:], gidx1_d[h].ap())
            gidx1_t.append(g)
            r = const.tile(list(ridx_d[h].shape), I16, name=f"ridxh{h}_t")
            nc.sync.dma_start(r[:], ridx_d[h].ap())
            ridx_t.append(r)
        pen_t = const.tile([1, BL], F32, name="pen_t")
        nc.sync.dma_start(pen_t[:], pen_d.ap())
        ones1_t = const.tile([1, NSEG16], F32, name="ones1_t")
        nc.sync.dma_start(ones1_t[:], ones1_d.ap())
        an3_t = const.tile([2, NSEG16], F32, name="an3_t")
        nc.sync.dma_start(an3_t[:], an3_d.ap())
        bl3_t = const.tile([2, BL], F32, name="bl3_t")
        nc.sync.dma_start(bl3_t[:], bl3_d.ap())

        # Augmented-row builder: dst rows get [value_r; value_res] at vrows
        # and +-ones at orows (value = src row0, ones = src row1).  The f32r
        # rounding happens in a [128, X] wide layout (fast), the residual is
        # staged back row-major, and one whole-tile f32r copy per chunk is
        # the rounding producer walrus wants.
        def build_aug(dst, src_d, width, vrows, orows, soff=0):
            """dst[4, width] f32r gets [value_r; value_res] at vrows and
            +-ones at orows, from src_d rows [value; ones] cols
            [soff, soff+width).  Rounding runs in a [128, X] wide layout;
            the whole-tile f32r copy per chunk is the rounding producer."""
            for c0i in range(0, width, 2048):
                cw = min(2048, width - c0i)
                assert cw % 128 == 0
                X = cw // 128
                c0 = c0i + soff
                wide_ap = src_d.ap()[0:1, c0:c0 + cw].rearrange(
                    "o (p x) -> (o p) x", p=128)
                vr = work.tile([128, 16], F32, name="vr", tag="augv", bufs=2)
                nc.sync.dma_start(vr[:, 0:X], wide_ap)
                vt1 = work.tile([128, 16], F32R, name="vt1", tag="augt", bufs=2)
                nc.vector.tensor_copy(vt1[:, 0:X], vr[:, 0:X])
                vres = work.tile([128, 16], F32, name="vres", tag="augr", bufs=2)
                nc.vector.tensor_sub(vres[:, 0:X], vr[:, 0:X],
                                     vt1[:, 0:X].bitcast(F32))
                stg = work.tile([4, 2048], F32, name="stg", tag="augstg", bufs=1)
                nc.sync.dma_start(stg[vrows[0]:vrows[0] + 1, 0:cw],
                                  src_d.ap()[0:1, c0:c0 + cw])
                nc.sync.dma_start(stg[vrows[1]:vrows[1] + 1, 0:cw],
                                  vres[:, 0:X])
                for r_ in orows:
                    nc.sync.dma_start(stg[r_:r_ + 1, 0:cw],
                                      src_d.ap()[1:2, c0:c0 + cw])
                nc.vector.tensor_copy(dst[:, c0i:c0i + cw], stg[:, 0:cw])

        # A side: [a2_r; a2_res; 1; 1]
        a3r_t = const.tile([4, EN_LOC], F32R, name="a3_r")
        build_aug(a3r_t, a3_d, EN_LOC, vrows=(0, 1), orows=(2, 3))

        # ---------------- K-major features ----------------
        ak_ts = [const.tile([128, EN_LOC], F32R, name=f"ak{d}") for d in range(D2)]
        ank_ts = [const.tile([128, 128], F32, name=f"ank{d}") for d in range(D2)]
        blk_ts = [const.tile([128, BL], F32, name=f"blk{d}") for d in range(D2)]

        def build_kmajor(idx0, idx1, n_idx, dsts, scale, single=False,
                         idx_off=0, col_off=0, nb=256):
            for ch0 in range(0, n_idx, nb):
                w = min(nb, n_idx - ch0)
                nsub = max(w // 128, 1)
                g0 = work.tile([128, nsub, D], F32, name="g0", tag="gbuf0")
                nc.gpsimd.dma_gather(
                    g0[:], emb_d.ap(),
                    idx0[:, (idx_off + ch0) // 16:(idx_off + ch0 + w) // 16],
                    w, w, D)
                if not single:
                    g1 = work.tile([128, nsub, D], F32, name="g1", tag="gbuf1")
                    nc.gpsimd.dma_gather(
                        g1[:], emb_d.ap(),
                        idx1[:, (idx_off + ch0) // 16:(idx_off + ch0 + w) // 16],
                        w, w, D)
                for s in range(nsub):
                    for d_ in range(D2):
                        pst = ps1.tile([128, 128], F32, name="pst", tag="ps1")
                        nc.tensor.matmul(pst[:], g0[:, s, d_ * 128:(d_ + 1) * 128],
                                         ident_t[:], is_transpose=True,
                                         start=True, stop=single)
                        if not single:
                            nc.tensor.matmul(pst[:], g1[:, s, d_ * 128:(d_ + 1) * 128],
                                             ident_t[:], is_transpose=True,
                                             start=False, stop=True)
                        col0 = col_off + ch0 + s * 128
                        cw = min(128, col_off + n_idx - col0)
                        nc.scalar.mul(dsts[d_][:, col0:col0 + cw], pst[:, 0:cw], scale)

        assert NSEG16 <= 128
        build_kmajor(idx_t["qe0"], idx_t["qe1"], EN_LOC, ak_ts, 1.0)
        build_kmajor(idx_t["nid"], None, 128, ank_ts, 1.0, single=True)
        build_kmajor(idx_t["lid"], None, BL, blk_ts, 2.0, single=True)

        # score accumulation PSUM [NSEG16, BL]
        ps_mean = psacc.tile([NSEG16, BL], F32, name="ps_mean")
        ps_node = psacc.tile([NSEG16, BL], F32, name="ps_node")

        keep_t = const.tile([128, T_Q], F32, name="keep_t")

        rep_ctx = tc.For_i(0, reps, 1) if reps > 1 else None
        if rep_ctx is not None:
            rep_ctx.__enter__()

        # open the ps_mean accumulation with the absent-label penalty row
        # (start=True zeroes the whole [NSEG16, BL] region; everything else
        # accumulates onto it in Tile-tracked emission order)
        nc.tensor.matmul(ps_mean[:], ones1_t[:], pen_t[:], start=True,
                         stop=False, skip_group_check=True)

        # ---------------- main: two label halves ----------------
        for h, H in enumerate(halves):
            off, chs = H["off"], H["chunks"]
            # B-side K-major for this half
            bk_ts = [hpool.tile([128, ELH], F32R, name=f"bk{d}", tag=f"bk{d}")
                     for d in range(D2)]
            build_kmajor(idx_t["le0"], idx_t["le1"], ELH, bk_ts, 2.0,
                         idx_off=off)
            # mask half
            m1 = hpool.tile([128, ELH], F32, name="m1", tag="m1")
            nc.sync.dma_start(m1[:], mask1_d.ap()[:, off:off + ELH])
            # B-side augmented rows for this half: [-1; -1; -b2_r; -b2_res]
            b3r_t = hpool.tile([4, ELH], F32R, name="b3_r", tag="b3r")
            build_aug(b3r_t, b3_d, ELH, vrows=(2, 3), orows=(0, 1), soff=off)

            # ---- orientation 1 ----
            l0, wseg, wpad = H["l0"], H["wseg"], H["wpad"]
            for t in range(T_Q):
                strip = work.tile([128, GWmax], F32, name="strip", tag="strip")
                prev = None
                for ch in chs:
                    c0l = ch["c0"] - off
                    w, s0, npad = ch["w"], ch["s0"], ch["npad"]
                    ps = ps1.tile([128, 512], F32, name="pso1", tag="ps1")
                    for d_ in range(D2):
                        nc.tensor.matmul(ps[:, 0:w],
                                         ak_ts[d_][:, t * 128:(t + 1) * 128],
                                         bk_ts[d_][:, c0l:c0l + w],
                                         start=(d_ == 0), stop=False)
                    nc.tensor.matmul(ps[:, 0:w], a3r_t[:, t * 128:(t + 1) * 128],
                                     b3r_t[:, c0l:c0l + w], start=False, stop=True)
                    sc = work.tile([128, 512], F32, name="sc1", tag="sc1")
                    if prev is None:
                        init = keep_t[:, t:t + 1] if h == 1 else NEG
                    else:
                        init = prev[:, prev_w - 1:prev_w]
                    nc.vector.tensor_tensor_scan(
                        sc[:, 0:w], m1[:, c0l:c0l + w], ps[:, 0:w], initial=init,
                        op0=mybir.AluOpType.add, op1=mybir.AluOpType.max)
                    nc.gpsimd.ap_gather(
                        strip[:, s0:s0 + npad], sc[:, 0:w],
                        gidx1_t[h][:, s0 // 16:(s0 + npad) // 16],
                        channels=128, num_elems=w, d=1, num_idxs=npad)
                    prev, prev_w = sc, w
                if h == 0:
                    nc.vector.tensor_copy(keep_t[:, t:t + 1],
                                          prev[:, prev_w - 1:prev_w])
                # slots -> label-id columns, clamp+sqrt, mean matmul
                go = work.tile([128, WPmax], F32, name="go1", tag="go1")
                nc.gpsimd.ap_gather(go[:, 0:wpad], strip[:, 0:H["GWH"]],
                                    ridx_t[h][:], channels=128,
                                    num_elems=H["GWH"], d=1, num_idxs=wpad)
                nc.vector.tensor_scalar(go[:, 0:wseg], go[:, 0:wseg], -0.25,
                                        1e-12, op0=mybir.AluOpType.mult,
                                        op1=mybir.AluOpType.max)
                nc.scalar.sqrt(go[:, 0:wseg], go[:, 0:wseg])
                wn = work.tile([128, NSEG16], F32, name="wn", tag="wn")
                nc.sync.dma_start(wn[:], wn_d.ap()[t * 128:(t + 1) * 128, :])
                for n0 in range(0, wseg, 512):
                    nw = min(512, wseg - n0)
                    nc.tensor.matmul(ps_mean[:, l0 + n0:l0 + n0 + nw], wn[:],
                                     go[:, n0:n0 + nw], start=False,
                                     stop=False, skip_group_check=True)

            # ---- orientation 2 ----
            HQ = EN_LOC // 2
            for m in range(ELT_H):
                sc2 = work.tile([128, EN_LOC], F32, name="sc2", tag="sc2")
                for hq in range(2):
                    psb = ps2.tile([128, HQ], F32, name="pso2", tag="ps2")
                    h0 = hq * HQ
                    for n0 in range(0, HQ, 512):
                        nw = min(512, HQ - n0)
                        for d_ in range(D2):
                            nc.tensor.matmul(psb[:, n0:n0 + nw],
                                             bk_ts[d_][:, m * 128:(m + 1) * 128],
                                             ak_ts[d_][:, h0 + n0:h0 + n0 + nw],
                                             start=(d_ == 0), stop=False)
                        nc.tensor.matmul(psb[:, n0:n0 + nw],
                                         b3r_t[:, m * 128:(m + 1) * 128],
                                         a3r_t[:, h0 + n0:h0 + n0 + nw],
                                         start=False, stop=True)
                    nc.vector.tensor_tensor_scan(
                        sc2[:, h0:h0 + HQ], mask2_t[:, h0:h0 + HQ], psb[:],
                        initial=(NEG if hq == 0 else sc2[:, h0 - 1:h0]),
                        op0=mybir.AluOpType.add, op1=mybir.AluOpType.max)
                g2 = work.tile([128, NSEG16], F32, name="g2", tag="g2")
                nc.gpsimd.ap_gather(g2[:], sc2[:, 0:EN_LOC], idx_t["gidx2"][:],
                                    channels=128, num_elems=EN_LOC, d=1,
                                    num_idxs=NSEG16)
                nc.vector.tensor_scalar(g2[:], g2[:], -0.25, 1e-12,
                                        op0=mybir.AluOpType.mult,
                                        op1=mybir.AluOpType.max)
                nc.scalar.sqrt(g2[:], g2[:])
                wl = work.tile([128, 1, BLV], F32, name="wl", tag="wl")
                mg = off // 128 + m
                nc.gpsimd.dma_gather(wl[:], vdiag_d.ap(),
                                     idx_t["lsegw"][:, mg * 8:(mg + 1) * 8],
                                     128, 128, BLV)
                nc.tensor.matmul(ps_mean[:], g2[:], wl[:, 0, 0:BL],
                                 start=False,
                                 stop=(h == 1 and m == ELT_H - 1),
                                 skip_group_check=True)

        # ---------------- node-level score + combine + topk ----------------
        for d_ in range(D2):
            nc.tensor.matmul(ps_node[:], ank_ts[d_][:, 0:NSEG16], blk_ts[d_][:],
                             start=(d_ == 0), stop=False)
        nc.tensor.matmul(ps_node[:], an3_t[:], bl3_t[:], start=False, stop=True)
        nd = work.tile([NSEG16, BL], F32, name="nd", bufs=1)
        nc.vector.tensor_scalar(nd[:], ps_node[:], -1.0, 1e-12,
                                op0=mybir.AluOpType.mult, op1=mybir.AluOpType.max)
        nc.scalar.sqrt(nd[:], nd[:])
        score_t = work.tile([NSEG16, BL], F32, name="score_t", bufs=1)
        nc.vector.scalar_tensor_tensor(
            score_t[:], nd[:], -(ALPHA / T), ps_mean[:],
            op0=mybir.AluOpType.mult, op1=mybir.AluOpType.add)
        nc.sync.dma_start(score_d.ap(), score_t[:])

        top_t = work.tile([NSEG16, 16], F32, name="top_t", bufs=1)
        tix_t = work.tile([NSEG16, 16], U32, name="tix_t", bufs=1)
        sc_copy = work.tile([NSEG16, BL], F32, name="sc_copy", bufs=1)
        nc.vector.tensor_copy(sc_copy[:], score_t[:])
        nc.vector.max(top_t[:, 0:8], sc_copy[:])
        nc.vector.max_index(tix_t[:, 0:8], top_t[:, 0:8], sc_copy[:])
        nc.vector.match_replace(sc_copy[:], top_t[:, 0:8], sc_copy[:], NEG)
        nc.vector.max(top_t[:, 8:16], sc_copy[:])
        nc.vector.max_index(tix_t[:, 8:16], top_t[:, 8:16], sc_copy[:])
        nc.sync.dma_start(tidx_d.ap(), tix_t[:])

        if rep_ctx is not None:
            rep_ctx.__exit__(None, None, None)

    nc.compile()
    return nc


def kernel(**inputs):
    p = _plan(inputs)
    nc = _build(p)
    in_maps = [c["inputs"] for c in p["per_core"]]
    res = run_bass_kernel_spmd(nc, in_maps, list(range(p["NC"])))
    return _assemble(p, [res.results[k] for k in range(p["NC"])])


def _assemble(p, results):
    BN, BL = p["BN"], p["BL"]
    score = np.zeros((BN, BL), np.float32)
    tidx = np.zeros((BN, TOPK), np.int32)
    for k, c in enumerate(p["per_core"]):
        r = results[k]
        n = c["nsegk"]
        score[c["gs0"]:c["gs1"]] = r["score"][:n]
        tidx[c["gs0"]:c["gs1"]] = r["tidx"][:n, :TOPK].astype(np.int32)
        pq = c["present_q"][:n]
        if not pq.all():
            rows = np.nonzero(~pq)[0] + c["gs0"]
            score[rows] = -np.inf
            tidx[rows] = np.arange(TOPK)[None, :]
    if len(p["absent_l"]):
        score[:, p["absent_l"]] = -np.inf
    return score, tidx


if __name__ == "__main__":
    sys.path.insert(0, "/root/problem")
    import reference

    inputs = {k: np.asarray(v) for k, v in reference.setup_inputs().items()}
    out = kernel(**inputs)
    print("score", out[0].shape, "tidx", out[1].shape)


# revision 27
# speedup vs baseline: 2.5846x; 1.8681x over previous
"""ExplainGNN retrieval kernel for 8 Trainium2 NeuronCores.

Strategy (per sharding hint): shard query edges (rows of the big
edge-distance matrix) across the 8 cores, split at node-segment
boundaries so every node segment lives on exactly one core. Each core:

  - gathers the embedding rows of its edges from a host-compacted table
    (device-side indirect DMA), builds edge features ef' = e_u + e_v in
    K-major layout via PE-transpose accumulation; the label side is
    processed in NPIECE pieces with double-buffered rebuild so the
    gather/transpose pipeline overlaps compute,
  - computes -d2' = 2*ef_n'.ef_l' - |ef_n'|^2 - |ef_l'|^2 via f32r
    matmuls (augmented K-chunk carries the norms as f32r value+residual
    pairs so their precision survives),
  - orientation 1 (query rows on partitions): segmented max over label
    segments via chained tensor_tensor_scan + ap_gather of endpoints,
  - orientation 2 (label rows on partitions, second matmul): segmented
    max over local node segments the same way,
  - per-piece batched tails: sqrt + segment means via small fp32 matmuls
    (accumulating into a [NSEG, BL] PSUM score tile) are emitted after
    the piece's matmul stream so cross-engine latency chains stay off
    the PE's in-order path,
  - node-level cdist in plain fp32 (exact; it dominates the score),
  - score = -(ALPHA/T)*d_node + mean terms; top-16 via max8/max_index/
    match_replace; host takes top-10.

Host reassembles per-core [NSEG, BL] score rows into [BN, BL] and the
top-k indices, and patches the (measure-zero) empty-segment edge cases
to match jax segment_max/-inf semantics exactly.
"""

import sys

sys.path.insert(0, "/opt/trn_rl_repo")

from contextlib import ExitStack

import numpy as np

import concourse.bass as bass
import concourse.tile as tile
from concourse import bacc, mybir
from concourse.bass_utils import run_bass_kernel_spmd

F32 = mybir.dt.float32
F32R = mybir.dt.float32r
I16 = mybir.dt.int16
U32 = mybir.dt.uint32

ALPHA = 0.5
T = 0.5
TOPK = 10
NCORES = 8
NEG = -3.0e38
PEN = -1.0e30
NPIECE = 4


def _ceil(a, b):
    return -(-a // b) * b


def wrap16(idx, parts=128):
    """16-partition-wrapped index layout for ap_gather / dma_gather,
    replicated across all partition groups."""
    idx = np.asarray(idx, np.int16)
    n = len(idx)
    assert n % 16 == 0, n
    tbl = np.zeros((16, n // 16), np.int16)
    tbl[np.arange(n) % 16, np.arange(n) // 16] = idx
    return np.tile(tbl, (parts // 16, 1))


def _plan(inputs, ncores=NCORES, npiece=NPIECE):
    emb = np.asarray(inputs["embedding"], np.float32)
    node_ids = np.asarray(inputs["node_ids"], np.int64)
    label_ids = np.asarray(inputs["label_ids"], np.int64)
    node_edge = np.asarray(inputs["node_edge"], np.int64)
    label_edge = np.asarray(inputs["label_edge"], np.int64)
    node_seg = np.asarray(inputs["node_seg"], np.int64)
    label_seg = np.asarray(inputs["label_seg"], np.int64)

    NN, D = emb.shape
    EN = node_edge.shape[1]
    EL = label_edge.shape[1]
    BN = node_ids.shape[0]
    BL = label_ids.shape[0]
    assert D % 128 == 0 and EL % (128 * npiece) == 0

    p = dict(D=D, EN=EN, EL=EL, BN=BN, BL=BL, NC=ncores, NP=npiece)

    # ---------------- query-edge split at segment boundaries ----------------
    cuts = [0]
    for k in range(1, ncores):
        c = (EN * k) // ncores
        while 0 < c < EN and node_seg[c] == node_seg[c - 1]:
            c += 1
        cuts.append(min(c, EN))
    cuts.append(EN)
    e0s, e1s = cuts[:-1], cuts[1:]
    gs0s = [0] + [int(node_seg[e]) if e < EN else BN for e in e1s[:-1]]
    gs1s = gs0s[1:] + [BN]

    EN_LOC = max(_ceil(max(e1 - e0 for e0, e1 in zip(e0s, e1s)), 256), 256)
    T_Q = EN_LOC // 128
    NSEG16 = _ceil(max(g1 - g0 for g0, g1 in zip(gs0s, gs1s)), 16)
    assert NSEG16 <= 128
    p.update(EN_LOC=EN_LOC, T_Q=T_Q, NSEG16=NSEG16, gs0s=gs0s, gs1s=gs1s)

    # ---------------- label pieces + fixed 512 chunks -----------------------
    seg_start = np.r_[0, 1 + np.nonzero(np.diff(label_seg))[0]]
    seg_end = np.r_[seg_start[1:], EL]
    seg_ids = label_seg[seg_start]          # present label segs (sorted)
    nseg_l = len(seg_ids)
    cnt_l = (seg_end - seg_start).astype(np.float32)
    coef_neigh = (1.0 - ALPHA) * 0.5 / T

    ELP = EL // npiece
    pieces = []
    for h in range(npiece):
        off = h * ELP
        chs = []
        slot = 0
        gcols = []
        for c0 in range(off, off + ELP, 512):
            w = min(512, off + ELP - c0)
            mask = (seg_end - 1 >= c0) & (seg_end - 1 < c0 + w)
            ends = seg_end[mask] - 1 - c0
            ne = len(ends)
            npad = max(_ceil(ne, 16), 16)
            ends_pad = np.concatenate([ends, np.zeros(npad - ne, np.int64)])
            gcols.append(wrap16(ends_pad))
            chs.append(dict(c0=c0, w=w, s0=slot, ne=ne, npad=npad,
                            segs=np.nonzero(mask)[0]))
            slot += npad
        pieces.append(dict(off=off, chunks=chs, GWH=slot,
                           gidx1=np.concatenate(gcols, axis=1)))
    seg_piece = np.zeros(nseg_l, np.int64)
    seg_slot = np.zeros(nseg_l, np.int64)
    for h, H in enumerate(pieces):
        for ch in H["chunks"]:
            for r, sidx in enumerate(ch["segs"]):
                seg_piece[sidx] = h
                seg_slot[sidx] = ch["s0"] + r
    # label-id regions per piece (ending segments form consecutive prefixes)
    bnd = [0]
    for h in range(npiece):
        n_end = int((seg_piece <= h).sum())
        bnd.append(int(seg_ids[n_end - 1]) + 1 if n_end else bnd[-1])
    bnd[-1] = BL
    for h, H in enumerate(pieces):
        l0, l1 = bnd[h], bnd[h + 1]
        wseg = max(l1 - l0, 1)
        wpad = max(_ceil(wseg, 16), 16)
        ridx = np.zeros(wpad, np.int64)
        for sidx in range(nseg_l):
            if seg_piece[sidx] == h and l0 <= seg_ids[sidx] < l0 + wseg:
                ridx[seg_ids[sidx] - l0] = seg_slot[sidx]
        H.update(l0=l0, wseg=wseg, wseg0=l1 - l0, wpad=wpad,
                 ridx_w=wrap16(ridx.astype(np.int16)))
    GWmax = max(H["GWH"] for H in pieces)
    WPmax = max(H["wpad"] for H in pieces)
    p.update(pieces=pieces, GWmax=GWmax, WPmax=WPmax)

    mask1 = np.zeros(EL, np.float32)
    mask1[seg_start] = NEG

    BLV = _ceil(BL, 64)
    Vdiag = np.zeros((BL, BLV), np.float32)
    Vdiag[seg_ids, seg_ids] = -coef_neigh / cnt_l

    absent_l = np.setdiff1d(np.arange(BL), seg_ids)
    pen = np.zeros(BL, np.float32)
    pen[absent_l] = PEN
    p.update(absent_l=absent_l, BLV=BLV)

    # ---------------- compaction + per-core tables --------------------------
    per_core, UMAXES, core_tmp = [], [], []
    for k in range(ncores):
        e0, e1 = e0s[k], e1s[k]
        gs0, gs1 = gs0s[k], gs1s[k]
        enk = e1 - e0
        qe0 = node_edge[0, e0:e1]
        qe1 = node_edge[1, e0:e1]
        if EN_LOC - enk:
            qe0 = np.r_[qe0, np.full(EN_LOC - enk, qe0[-1] if enk else 0)]
            qe1 = np.r_[qe1, np.full(EN_LOC - enk, qe1[-1] if enk else 0)]
        nid = np.r_[node_ids[gs0:gs1], np.zeros(128 - (gs1 - gs0), np.int64)]
        ids = np.concatenate([label_edge[0], label_edge[1], qe0, qe1, nid,
                              label_ids])
        uniq, inv = np.unique(ids, return_inverse=True)
        UMAXES.append(len(uniq))
        core_tmp.append((e0, e1, gs0, gs1, qe0, qe1, nid, uniq, inv))
    UMAX = _ceil(max(UMAXES), 128)
    assert UMAX <= 32000, UMAX
    p.update(UMAX=UMAX)

    lseg_w = wrap16(label_seg.astype(np.int16))

    for k in range(ncores):
        e0, e1, gs0, gs1, qe0, qe1, nid, uniq, inv = core_tmp[k]
        enk = e1 - e0
        nsegk = gs1 - gs0
        emb_loc = np.zeros((UMAX, D), np.float32)
        emb_loc[: len(uniq)] = emb[uniq]
        n = 0
        le0c = inv[n:n + EL]; n += EL
        le1c = inv[n:n + EL]; n += EL
        qe0c = inv[n:n + EN_LOC]; n += EN_LOC
        qe1c = inv[n:n + EN_LOC]; n += EN_LOC
        nidc = inv[n:n + 128]; n += 128
        lidc = inv[n:n + BL]

        ef_q = emb[qe0] + emb[qe1]
        a2 = np.einsum("ij,ij->i", ef_q, ef_q).astype(np.float32)
        ef_l = emb[label_edge[0]] + emb[label_edge[1]]
        b2 = np.einsum("ij,ij->i", ef_l, ef_l).astype(np.float32)
        n2row = emb[nid[:NSEG16]]
        n2 = np.einsum("ij,ij->i", n2row, n2row).astype(np.float32)
        l2row = emb[label_ids]
        l2 = np.einsum("ij,ij->i", l2row, l2row).astype(np.float32)

        lseg_q = node_seg[e0:e1] - gs0
        q_start = np.r_[0, 1 + np.nonzero(np.diff(lseg_q))[0]] if enk else np.array([], np.int64)
        q_end = np.r_[q_start[1:], enk] if enk else np.array([], np.int64)
        q_ids = lseg_q[q_start] if enk else np.array([], np.int64)
        mask2 = np.zeros(EN_LOC, np.float32)
        if enk:
            mask2[q_start] = NEG
        if enk < EN_LOC:
            mask2[enk] = NEG
        gidx2_flat = np.zeros(NSEG16, np.int64)
        cnt_q = np.zeros(NSEG16, np.float32)
        present_q = np.zeros(NSEG16, bool)
        for s, e, sid in zip(q_start, q_end, q_ids):
            gidx2_flat[sid] = e - 1
            cnt_q[sid] = e - s
            present_q[sid] = True
        W_n = np.zeros((EN_LOC, NSEG16), np.float32)
        if enk:
            W_n[np.arange(enk), lseg_q] = -coef_neigh / cnt_q[lseg_q]

        im = {
            "emb_loc": emb_loc,
            "vdiag": Vdiag,
            "le0": wrap16(le0c.astype(np.int16)),
            "le1": wrap16(le1c.astype(np.int16)),
            "qe0": wrap16(qe0c.astype(np.int16)),
            "qe1": wrap16(qe1c.astype(np.int16)),
            "nid": wrap16(nidc.astype(np.int16)),
            "lid": wrap16(lidc.astype(np.int16)),
            "lsegw": lseg_w,
            "gidx2": wrap16(gidx2_flat.astype(np.int16)),
            "mask1": np.tile(mask1[None, :], (128, 1)),
            "mask2": np.tile(mask2[None, :], (128, 1)),
            "a3": np.stack([a2, np.ones(EN_LOC, np.float32)]),
            "b3": np.stack([-b2, -np.ones(EL, np.float32)]),
            "an3": np.stack([n2, np.ones(NSEG16, np.float32)]),
            "bl3": np.stack([-np.ones(BL, np.float32), -l2]),
            "pen": pen[None, :],
            "ones1": np.ones((1, NSEG16), np.float32),
            "wn": W_n,
            "ident": np.eye(128, dtype=np.float32),
        }
        for h, H in enumerate(pieces):
            im[f"gidx1h{h}"] = H["gidx1"]
            im[f"ridxh{h}"] = H["ridx_w"]
        per_core.append(dict(inputs=im, gs0=gs0, gs1=gs1, nsegk=nsegk,
                             present_q=present_q))
    p["per_core"] = per_core
    return p


def _build(p, reps=1, skip_o1=False, skip_o2=False, o2_scan_gpsimd=False):
    D, EL, BL, BLV = p["D"], p["EL"], p["BL"], p["BLV"]
    EN_LOC, T_Q, NSEG16, UMAX = p["EN_LOC"], p["T_Q"], p["NSEG16"], p["UMAX"]
    D2 = D // 128
    NP = p["NP"]
    ELP = EL // NP
    ELT_P = ELP // 128
    pieces = p["pieces"]
    GWmax, WPmax = p["GWmax"], p["WPmax"]

    nc = bacc.Bacc("TRN2", target_bir_lowering=False, debug=False,
                   num_devices=p["NC"])

    def din(name, shape, dt=F32):
        return nc.dram_tensor(name, shape, dt, kind="ExternalInput")

    emb_d = din("emb_loc", [UMAX, D])
    vdiag_d = din("vdiag", [BL, BLV])
    idx_d = {n: din(n, [128, s // 16], I16) for n, s in [
        ("le0", EL), ("le1", EL), ("qe0", EN_LOC), ("qe1", EN_LOC),
        ("nid", 128), ("lid", BL), ("lsegw", EL), ("gidx2", NSEG16)]}
    gidx1_d = [din(f"gidx1h{h}", [128, pieces[h]["GWH"] // 16], I16)
               for h in range(NP)]
    ridx_d = [din(f"ridxh{h}", [128, pieces[h]["wpad"] // 16], I16)
              for h in range(NP)]
    mask1_d = din("mask1", [128, EL])
    mask2_d = din("mask2", [128, EN_LOC])
    a3_d = din("a3", [2, EN_LOC])
    b3_d = din("b3", [2, EL])
    an3_d = din("an3", [2, NSEG16])
    bl3_d = din("bl3", [2, BL])
    pen_d = din("pen", [1, BL])
    ones1_d = din("ones1", [1, NSEG16])
    wn_d = din("wn", [EN_LOC, NSEG16])
    ident_d = din("ident", [128, 128])

    score_d = nc.dram_tensor("score", [NSEG16, BL], F32, kind="ExternalOutput")
    tidx_d = nc.dram_tensor("tidx", [NSEG16, 16], U32, kind="ExternalOutput")

    with tile.TileContext(nc) as tc, ExitStack() as ctx:
        const = ctx.enter_context(tc.tile_pool(name="const", bufs=1))
        hpool = ctx.enter_context(tc.tile_pool(name="hpool", bufs=2))
        work = ctx.enter_context(tc.tile_pool(name="work", bufs=2))
        ps1 = ctx.enter_context(tc.tile_pool(name="ps1", bufs=2, space="PSUM"))
        ps2 = ctx.enter_context(tc.tile_pool(name="ps2", bufs=2, space="PSUM"))
        psacc = ctx.enter_context(tc.tile_pool(name="psacc", bufs=1, space="PSUM"))

        # ---------------- constants / tables ----------------
        ident_t = const.tile([128, 128], F32, name="ident_t")
        nc.sync.dma_start(ident_t[:], ident_d.ap())
        mask2_t = const.tile([128, EN_LOC], F32, name="mask2_t")
        nc.sync.dma_start(mask2_t[:], mask2_d.ap())
        idx_t = {}
        for n, d_ in idx_d.items():
            idx_t[n] = const.tile(list(d_.shape), I16, name=f"{n}_t")
            nc.sync.dma_start(idx_t[n][:], d_.ap())
        gidx1_t, ridx_t = [], []
        for h in range(NP):
            g = const.tile(list(gidx1_d[h].shape), I16, name=f"gidx1h{h}_t")
            nc.sync.dma_start(g[:], gidx1_d[h].ap())
            gidx1_t.append(g)
            r = const.tile(list(ridx_d[h].shape), I16, name=f"ridxh{h}_t")
            nc.sync.dma_start(r[:], ridx_d[h].ap())
            ridx_t.append(r)
        pen_t = const.tile([1, BL], F32, name="pen_t")
        nc.sync.dma_start(pen_t[:], pen_d.ap())
        ones1_t = const.tile([1, NSEG16], F32, name="ones1_t")
        nc.sync.dma_start(ones1_t[:], ones1_d.ap())
        an3_t = const.tile([2, NSEG16], F32, name="an3_t")
        nc.sync.dma_start(an3_t[:], an3_d.ap())
        bl3_t = const.tile([2, BL], F32, name="bl3_t")
        nc.sync.dma_start(bl3_t[:], bl3_d.ap())
        wn_ts = []
        for t in range(T_Q):
            w = const.tile([128, NSEG16], F32, name=f"wn{t}_t")
            nc.sync.dma_start(w[:], wn_d.ap()[t * 128:(t + 1) * 128, :])
            wn_ts.append(w)

        def build_aug(dst, src_d, width, vrows, orows, soff=0):
            """dst[4, width] f32r gets [value_r; value_res] at vrows and
            +-ones at orows, from src_d rows [value; ones] cols
            [soff, soff+width)."""
            for c0i in range(0, width, 1024):
                cw = min(1024, width - c0i)
                assert cw % 128 == 0
                X = cw // 128
                c0 = c0i + soff
                wide_ap = src_d.ap()[0:1, c0:c0 + cw].rearrange(
                    "o (p x) -> (o p) x", p=128)
                vr = work.tile([128, 16], F32, name="vr", tag="augv", bufs=2)
                nc.sync.dma_start(vr[:, 0:X], wide_ap)
                vt1 = work.tile([128, 16], F32R, name="vt1", tag="augt", bufs=2)
                nc.vector.tensor_copy(vt1[:, 0:X], vr[:, 0:X])
                vres = work.tile([128, 16], F32, name="vres", tag="augr", bufs=2)
                nc.vector.tensor_sub(vres[:, 0:X], vr[:, 0:X],
                                     vt1[:, 0:X].bitcast(F32))
                stg = work.tile([4, 1024], F32, name="stg", tag="augstg", bufs=1)
                nc.sync.dma_start(stg[vrows[0]:vrows[0] + 1, 0:cw],
                                  src_d.ap()[0:1, c0:c0 + cw])
                nc.sync.dma_start(stg[vrows[1]:vrows[1] + 1, 0:cw],
                                  vres[:, 0:X])
                for r_ in orows:
                    nc.sync.dma_start(stg[r_:r_ + 1, 0:cw],
                                      src_d.ap()[1:2, c0:c0 + cw])
                nc.vector.tensor_copy(dst[:, c0i:c0i + cw], stg[:, 0:cw])

        a3r_t = const.tile([4, EN_LOC], F32R, name="a3_r")
        build_aug(a3r_t, a3_d, EN_LOC, vrows=(0, 1), orows=(2, 3))

        # ---------------- K-major features ----------------
        ak_ts = [const.tile([128, EN_LOC], F32R, name=f"ak{d}") for d in range(D2)]
        ank_ts = [const.tile([128, 128], F32, name=f"ank{d}") for d in range(D2)]
        blk_ts = [const.tile([128, BL], F32, name=f"blk{d}") for d in range(D2)]

        def build_kmajor(idx0, idx1, n_idx, dsts, scale, single=False,
                         idx_off=0, nb=256):
            for ch0 in range(0, n_idx, nb):
                w = min(nb, n_idx - ch0)
                nsub = max(w // 128, 1)
                g0 = work.tile([128, nsub, D], F32, name="g0", tag="gbuf0")
                nc.gpsimd.dma_gather(
                    g0[:], emb_d.ap(),
                    idx0[:, (idx_off + ch0) // 16:(idx_off + ch0 + w) // 16],
                    w, w, D)
                if not single:
                    g1 = work.tile([128, nsub, D], F32, name="g1", tag="gbuf1")
                    nc.gpsimd.dma_gather(
                        g1[:], emb_d.ap(),
                        idx1[:, (idx_off + ch0) // 16:(idx_off + ch0 + w) // 16],
                        w, w, D)
                for s in range(nsub):
                    for d_ in range(D2):
                        pst = ps1.tile([128, 128], F32, name="pst", tag="ps1")
                        nc.tensor.matmul(pst[:], g0[:, s, d_ * 128:(d_ + 1) * 128],
                                         ident_t[:], is_transpose=True,
                                         start=True, stop=single)
                        if not single:
                            nc.tensor.matmul(pst[:], g1[:, s, d_ * 128:(d_ + 1) * 128],
                                             ident_t[:], is_transpose=True,
                                             start=False, stop=True)
                        col0 = ch0 + s * 128
                        cw = min(128, n_idx - col0)
                        nc.scalar.mul(dsts[d_][:, col0:col0 + cw], pst[:, 0:cw],
                                      scale)

        assert NSEG16 <= 128
        build_kmajor(idx_t["qe0"], idx_t["qe1"], EN_LOC, ak_ts, 1.0)
        build_kmajor(idx_t["nid"], None, 128, ank_ts, 1.0, single=True)
        build_kmajor(idx_t["lid"], None, BL, blk_ts, 2.0, single=True)

        ps_mean = psacc.tile([NSEG16, BL], F32, name="ps_mean")
        ps_node = psacc.tile([NSEG16, BL], F32, name="ps_node")
        keep_t = const.tile([128, T_Q], F32, name="keep_t")

        rep_ctx = tc.For_i(0, reps, 1) if reps > 1 else None
        if rep_ctx is not None:
            rep_ctx.__enter__()

        # open the ps_mean accumulation with the absent-label penalty row
        # (start=True zeroes the whole [NSEG16, BL]; everything else
        # accumulates in Tile-tracked emission order)
        nc.tensor.matmul(ps_mean[:], ones1_t[:], pen_t[:], start=True,
                         stop=skip_o2, skip_group_check=True)

        o2_scan = nc.gpsimd if o2_scan_gpsimd else nc.vector

        # ---------------- main: NP label pieces ----------------
        for h, H in enumerate(pieces):
            off, chs = H["off"], H["chunks"]
            l0, wseg, wpad = H["l0"], H["wseg"], H["wpad"]
            # B-side K-major rebuild (double-buffered across pieces)
            bk_ts = [hpool.tile([128, ELP], F32R, name=f"bk{d}", tag=f"bk{d}")
                     for d in range(D2)]
            build_kmajor(idx_t["le0"], idx_t["le1"], ELP, bk_ts, 2.0,
                         idx_off=off)
            m1 = hpool.tile([128, ELP], F32, name="m1", tag="m1", bufs=1)
            nc.sync.dma_start(m1[:], mask1_d.ap()[:, off:off + ELP])
            b3r_t = hpool.tile([4, ELP], F32R, name="b3_r", tag="b3r")
            build_aug(b3r_t, b3_d, ELP, vrows=(2, 3), orows=(0, 1), soff=off)

            # ---- orientation 1: matmul + scan + endpoint gathers ----
            go_ts = []
            for t in range(T_Q if not skip_o1 else 0):
                strip = work.tile([128, GWmax], F32, name="strip", tag="strip")
                prev = None
                for ch in chs:
                    c0l = ch["c0"] - off
                    w, s0, npad = ch["w"], ch["s0"], ch["npad"]
                    ps = ps1.tile([128, 512], F32, name="pso1", tag="ps1")
                    for d_ in range(D2):
                        nc.tensor.matmul(ps[:, 0:w],
                                         ak_ts[d_][:, t * 128:(t + 1) * 128],
                                         bk_ts[d_][:, c0l:c0l + w],
                                         start=(d_ == 0), stop=False)
                    nc.tensor.matmul(ps[:, 0:w], a3r_t[:, t * 128:(t + 1) * 128],
                                     b3r_t[:, c0l:c0l + w], start=False, stop=True)
                    sc = work.tile([128, 512], F32, name="sc1", tag="sc1",
                                   bufs=2)
                    if prev is None:
                        init = keep_t[:, t:t + 1] if h > 0 else NEG
                    else:
                        init = prev[:, prev_w - 1:prev_w]
                    nc.vector.tensor_tensor_scan(
                        sc[:, 0:w], m1[:, c0l:c0l + w], ps[:, 0:w], initial=init,
                        op0=mybir.AluOpType.add, op1=mybir.AluOpType.max)
                    nc.gpsimd.ap_gather(
                        strip[:, s0:s0 + npad], sc[:, 0:w],
                        gidx1_t[h][:, s0 // 16:(s0 + npad) // 16],
                        channels=128, num_elems=w, d=1, num_idxs=npad)
                    prev, prev_w = sc, w
                if h < NP - 1:
                    nc.vector.tensor_copy(keep_t[:, t:t + 1],
                                          prev[:, prev_w - 1:prev_w])
                go = work.tile([128, WPmax], F32, name=f"go{t}", tag=f"go{t}")
                nc.gpsimd.ap_gather(go[:, 0:wpad], strip[:, 0:H["GWH"]],
                                    ridx_t[h][:], channels=128,
                                    num_elems=H["GWH"], d=1, num_idxs=wpad)
                go_ts.append(go)

            # ---- orientation 2: matmul + scan + endpoint gather ----
            HQ = EN_LOC // 2
            g2_ts = []
            for m in range(ELT_P if not skip_o2 else 0):
                sc2 = work.tile([128, EN_LOC], F32, name="sc2", tag="sc2")
                for hq in range(2):
                    psb = ps2.tile([128, HQ], F32, name="pso2", tag="ps2")
                    h0 = hq * HQ
                    for n0 in range(0, HQ, 512):
                        nw = min(512, HQ - n0)
                        for d_ in range(D2):
                            nc.tensor.matmul(psb[:, n0:n0 + nw],
                                             bk_ts[d_][:, m * 128:(m + 1) * 128],
                                             ak_ts[d_][:, h0 + n0:h0 + n0 + nw],
                                             start=(d_ == 0), stop=False)
                        nc.tensor.matmul(psb[:, n0:n0 + nw],
                                         b3r_t[:, m * 128:(m + 1) * 128],
                                         a3r_t[:, h0 + n0:h0 + n0 + nw],
                                         start=False, stop=True)
                    o2_scan.tensor_tensor_scan(
                        sc2[:, h0:h0 + HQ], mask2_t[:, h0:h0 + HQ], psb[:],
                        initial=(NEG if hq == 0 else sc2[:, h0 - 1:h0]),
                        op0=mybir.AluOpType.add, op1=mybir.AluOpType.max)
                g2 = work.tile([128, NSEG16], F32, name=f"g2_{m}", tag=f"g2_{m}")
                nc.gpsimd.ap_gather(g2[:], sc2[:, 0:EN_LOC], idx_t["gidx2"][:],
                                    channels=128, num_elems=EN_LOC, d=1,
                                    num_idxs=NSEG16)
                g2_ts.append(g2)

            # ---- batched tails: clamp+sqrt then mean matmuls ----
            for go in go_ts:
                nc.vector.tensor_scalar(go[:, 0:wseg], go[:, 0:wseg], -0.25,
                                        1e-12, op0=mybir.AluOpType.mult,
                                        op1=mybir.AluOpType.max)
                nc.scalar.sqrt(go[:, 0:wseg], go[:, 0:wseg])
            for g2 in g2_ts:
                nc.vector.tensor_scalar(g2[:], g2[:], -0.25, 1e-12,
                                        op0=mybir.AluOpType.mult,
                                        op1=mybir.AluOpType.max)
                nc.scalar.sqrt(g2[:], g2[:])
            for t, go in enumerate(go_ts):
                for n0 in range(0, H["wseg0"], 512):
                    nw = min(512, H["wseg0"] - n0)
                    nc.tensor.matmul(ps_mean[:, l0 + n0:l0 + n0 + nw],
                                     wn_ts[t][:], go[:, n0:n0 + nw],
                                     start=False, stop=False,
                                     skip_group_check=True)
            for m, g2 in enumerate(g2_ts):
                wl = work.tile([128, 1, BLV], F32, name="wl", tag="wl", bufs=3)
                mg = off // 128 + m
                nc.gpsimd.dma_gather(wl[:], vdiag_d.ap(),
                                     idx_t["lsegw"][:, mg * 8:(mg + 1) * 8],
                                     128, 128, BLV)
                nc.tensor.matmul(ps_mean[:], g2[:], wl[:, 0, 0:BL],
                                 start=False,
                                 stop=(h == NP - 1 and m == len(g2_ts) - 1),
                                 skip_group_check=True)

        # ---------------- node-level score + combine + topk ----------------
        for d_ in range(D2):
            nc.tensor.matmul(ps_node[:], ank_ts[d_][:, 0:NSEG16], blk_ts[d_][:],
                             start=(d_ == 0), stop=False)
        nc.tensor.matmul(ps_node[:], an3_t[:], bl3_t[:], start=False, stop=True)
        nd = work.tile([NSEG16, BL], F32, name="nd", bufs=1)
        nc.vector.tensor_scalar(nd[:], ps_node[:], -1.0, 1e-12,
                                op0=mybir.AluOpType.mult, op1=mybir.AluOpType.max)
        nc.scalar.sqrt(nd[:], nd[:])
        score_t = work.tile([NSEG16, BL], F32, name="score_t", bufs=1)
        nc.vector.scalar_tensor_tensor(
            score_t[:], nd[:], -(ALPHA / T), ps_mean[:],
            op0=mybir.AluOpType.mult, op1=mybir.AluOpType.add)
        nc.sync.dma_start(score_d.ap(), score_t[:])

        top_t = work.tile([NSEG16, 16], F32, name="top_t", bufs=1)
        tix_t = work.tile([NSEG16, 16], U32, name="tix_t", bufs=1)
        sc_copy = work.tile([NSEG16, BL], F32, name="sc_copy", bufs=1)
        nc.vector.tensor_copy(sc_copy[:], score_t[:])
        nc.vector.max(top_t[:, 0:8], sc_copy[:])
        nc.vector.max_index(tix_t[:, 0:8], top_t[:, 0:8], sc_copy[:])
        nc.vector.match_replace(sc_copy[:], top_t[:, 0:8], sc_copy[:], NEG)
        nc.vector.max(top_t[:, 8:16], sc_copy[:])
        nc.vector.max_index(tix_t[:, 8:16], top_t[:, 8:16], sc_copy[:])
        nc.sync.dma_start(tidx_d.ap(), tix_t[:])

        if rep_ctx is not None:
            rep_ctx.__exit__(None, None, None)

    nc.compile()
    return nc


def kernel(**inputs):
    p = _plan(inputs)
    nc = _build(p)
    in_maps = [c["inputs"] for c in p["per_core"]]
    res = run_bass_kernel_spmd(nc, in_maps, list(range(p["NC"])))
    return _assemble(p, [res.results[k] for k in range(p["NC"])])


def _assemble(p, results):
    BN, BL = p["BN"], p["BL"]
    score = np.zeros((BN, BL), np.float32)
    tidx = np.zeros((BN, TOPK), np.int32)
    for k, c in enumerate(p["per_core"]):
        r = results[k]
        n = c["nsegk"]
        score[c["gs0"]:c["gs1"]] = r["score"][:n]
        tidx[c["gs0"]:c["gs1"]] = r["tidx"][:n, :TOPK].astype(np.int32)
        pq = c["present_q"][:n]
        if not pq.all():
            rows = np.nonzero(~pq)[0] + c["gs0"]
            score[rows] = -np.inf
            tidx[rows] = np.arange(TOPK)[None, :]
    if len(p["absent_l"]):
        score[:, p["absent_l"]] = -np.inf
    return score, tidx


if __name__ == "__main__":
    sys.path.insert(0, "/root/problem")
    import reference

    inputs = {k: np.asarray(v) for k, v in reference.setup_inputs().items()}
    out = kernel(**inputs)
    print("score", out[0].shape, "tidx", out[1].shape)
